# revision 7
# baseline (speedup 1.0000x reference)
"""Trainium2 Bass kernel for nn_MAMoE (conv-MoE -> row attention -> MLP-MoE).

Sharding: 8 cores = (batch b in 0..3) x (H-half in 0..1). All routing is
per-token; the reference's swapaxes(1,2) means attention row r produces
output column w=r, so each core independently computes the full pipeline
for its 48 attention rows and the host reassembles along W.

Layout: padded row stride 104 (8 zero cols serve as both right halo of
row r and left halo of row r+1); conv/gate matmuls use strided rhs APs
([4 rows @ 104, 96]) so no pad columns are ever computed. scores use a
host-fused A = scale * Wq @ Wk^T so only one projection (qh) is needed.
bf16 everywhere with fp32 PSUM accumulation. Phase A is branch-interleaved
and software-pipelined two groups deep (scores of group g-1 and attention
tail of group g-2 are emitted under group g's convs) so the in-order PE
queue never blocks on the ACT/DVE softmax chain. Phase-B weights preload
on the second hardware DMA queue during Phase A.
"""
import numpy as np
import ml_dtypes

import concourse.bass as bass
import concourse.mybir as mybir
import concourse.tile as tile
from concourse import bacc
from concourse.bass_utils import run_bass_kernel_spmd
from concourse.masks import make_identity

F32 = mybir.dt.float32
F32R = mybir.dt.float32r
BF16 = mybir.dt.bfloat16
BF = ml_dtypes.bfloat16

B, HH, WW, C = 4, 96, 96, 384
HD = 128
SCALE = float((HD // 3) ** -0.5)  # 42**-0.5
N_CORES = 8
R = 48            # attention rows per core
RP = 66           # slack row + 8 halo + 48 + 8 halo + 1 slack row
SP = 104          # padded row stride (8 zero pad + 96 valid)
T = R * 96        # tokens per core = 4608
GROUPS = R // 4   # 12 groups of 4 rows
GN = 4 * 96       # tokens per group = 384
# MLP tiles: 8x512 then 2x256 (narrow tail shortens the end-of-kernel drain)
TILES = [(t * 512, 512) for t in range(8)] + [(4096, 256), (4352, 128), (4480, 128)]

TAPS_A = [
    [(dr, ds) for dr in (-1, 0, 1) for ds in (-1, 0, 1)],
    [(dr, 0) for dr in range(-4, 5)],
    [(0, ds) for ds in range(-4, 5)],
]
TAPS_B = [
    [(dr, ds) for dr in (-2, 0, 2) for ds in (-2, 0, 2)],
    [(dr, 0) for dr in range(-8, 9, 2)],
    [(0, ds) for ds in range(-8, 9, 2)],
]

_CACHED_NC = None


def build_kernel():
    nc = bacc.Bacc("TRN2", target_bir_lowering=False, debug=False)

    xp = nc.dram_tensor("xp", [C, RP, SP], BF16, kind="ExternalInput").ap()
    wca = nc.dram_tensor("wca", [3, HD, 9, HD], BF16, kind="ExternalInput").ap()
    wcb = nc.dram_tensor("wcb", [3, HD, 9, HD], BF16, kind="ExternalInput").ap()
    bca = nc.dram_tensor("bca", [HD, 3], F32, kind="ExternalInput").ap()
    bcb = nc.dram_tensor("bcb", [HD, 3], F32, kind="ExternalInput").ap()
    wgd = nc.dram_tensor("wgd", [3, HD, HD], BF16, kind="ExternalInput").ap()
    eb3 = nc.dram_tensor("eb3", [3, 384], BF16, kind="ExternalInput").ap()
    wqh = nc.dram_tensor("wqh", [3, HD, HD], BF16, kind="ExternalInput").ap()
    wv = nc.dram_tensor("wv", [3, HD, HD], BF16, kind="ExternalInput").ap()
    bap = nc.dram_tensor("bap", [HD, 3], F32, kind="ExternalInput").ap()
    wgf = nc.dram_tensor("wgf", [3, HD, HD], BF16, kind="ExternalInput").ap()
    w1 = nc.dram_tensor("w1", [3, HD, 3, 1536], BF16, kind="ExternalInput").ap()
    b1 = nc.dram_tensor("b1", [HD, 3, 12], F32, kind="ExternalInput").ap()
    w2 = nc.dram_tensor("w2", [3, HD, 12, C], BF16, kind="ExternalInput").ap()
    b2r = nc.dram_tensor("b2r", [3, C], BF16, kind="ExternalInput").ap()
    bpr = nc.dram_tensor("bpr", [HD, 3], F32, kind="ExternalInput").ap()
    out_cm = nc.dram_tensor("out_cm", [C, T], F32, kind="ExternalOutput").ap()

    with tile.TileContext(nc) as tc:
        with tc.tile_pool(name="consts", bufs=1) as consts, \
             tc.tile_pool(name="persist", bufs=1) as persist:
            ident = consts.tile([HD, HD], F32)
            make_identity(nc, ident)
            identb = consts.tile([HD, HD], BF16)
            nc.vector.tensor_copy(identb, ident)

            bca_sb = persist.tile([HD, 3], F32)
            bcb_sb = persist.tile([HD, 3], F32)
            bap_sb = persist.tile([HD, 3], F32)

            xc_t = [persist.tile([HD, T], BF16, tag=f"xc{i}", name=f"xc{i}") for i in range(3)]

            # Phase-B weights (DMAs issued later, on the scalar HWDGE queue)
            b1_sb = persist.tile([HD, 3, 12], F32)
            b2r_sb = persist.tile([3, C], BF16)
            wgf_sb = persist.tile([HD, 3, HD], BF16)
            bpr_sb = persist.tile([HD, 3], F32)
            eb3_sb = persist.tile([3, 384], BF16)
            w1_sb = [persist.tile([HD, 3, 1536], BF16, tag=f"w1_{e}", name=f"w1_{e}")
                     for e in range(3)]
            w2_sb = [persist.tile([HD, 12, C], BF16, tag=f"w2_{e}", name=f"w2_{e}")
                     for e in range(3)]

            phase_b_loads = []
            for e in range(3):
                phase_b_loads.append((w1_sb[e], w1[e]))
                phase_b_loads.append((w2_sb[e], w2[e]))
            phase_b_loads += [
                (b1_sb, b1), (b2r_sb, b2r), (bpr_sb, bpr), (eb3_sb, eb3),
            ]

            # ---------------- Phase A: conv MoE + attention, interleaved --
            with tc.tile_pool(name="xw", bufs=1) as xw, \
                 tc.tile_pool(name="gp3", bufs=3) as gp3, \
                 tc.tile_pool(name="gp6", bufs=6) as gp6, \
                 tc.tile_pool(name="ap3", bufs=3) as ap3, \
                 tc.tile_pool(name="ap9", bufs=9) as ap9, \
                 tc.tile_pool(name="psC", bufs=3, space="PSUM") as psC, \
                 tc.tile_pool(name="psT", bufs=5, space="PSUM") as psT:
                xp_sb = [xw.tile([HD, RP, SP], BF16, tag=f"xp{i}", name=f"xp{i}")
                         for i in range(3)]
                wgd_sb, wca_sb, wcb_sb, wqh_sb, wv_sb = [], [], [], [], []
                for i in range(3):
                    wgd_sb.append(xw.tile([HD, HD], BF16, tag=f"wgd{i}", name=f"wgd{i}"))
                    wca_sb.append(xw.tile([HD, 9, HD], BF16, tag=f"wca{i}", name=f"wca{i}"))
                    wcb_sb.append(xw.tile([HD, 9, HD], BF16, tag=f"wcb{i}", name=f"wcb{i}"))
                    wqh_sb.append(xw.tile([HD, HD], BF16, tag=f"wqh{i}", name=f"wqh{i}"))
                    wv_sb.append(xw.tile([HD, HD], BF16, tag=f"wv{i}", name=f"wv{i}"))

                def loadw(i, eng):
                    eng.dma_start(out=wgd_sb[i], in_=wgd[i])
                    eng.dma_start(out=wca_sb[i], in_=wca[i])
                    eng.dma_start(out=wcb_sb[i], in_=wcb[i])
                    eng.dma_start(out=wqh_sb[i], in_=wqh[i])
                    eng.dma_start(out=wv_sb[i], in_=wv[i])

                # Criticality-ordered DMA issue across both HWDGE queues.
                nc.sync.dma_start(out=xp_sb[0][:, :25, :], in_=xp[0:HD, :25, :])
                loadw(0, nc.sync)
                nc.scalar.dma_start(out=xp_sb[1][:, :25, :],
                                    in_=xp[HD:2 * HD, :25, :])
                nc.scalar.dma_start(out=xp_sb[2][:, :25, :],
                                    in_=xp[2 * HD:3 * HD, :25, :])
                nc.scalar.dma_start(out=bca_sb, in_=bca)
                nc.scalar.dma_start(out=bcb_sb, in_=bcb)
                nc.scalar.dma_start(out=bap_sb, in_=bap)
                loadw(1, nc.sync)
                loadw(2, nc.sync)
                for i in range(3):
                    nc.sync.dma_start(out=xp_sb[i][:, 25:45, :],
                                      in_=xp[i * HD:(i + 1) * HD, 25:45, :])
                for i in range(3):
                    nc.sync.dma_start(out=xp_sb[i][:, 45:, :],
                                      in_=xp[i * HD:(i + 1) * HD, 45:, :])
                nc.sync.dma_start(out=wgf_sb, in_=wgf.rearrange("a p b -> p a b"))

                xpf = [xp_sb[i].rearrange("p r s -> p (r s)") for i in range(3)]

                # p-state warm-up: keep PE busy (and ramping) while the
                # first input DMAs land; depends only on identb.
                warm = psT.tile([HD, HD], BF16, tag="ps")
                for _ in range(40):
                    nc.tensor.transpose(warm, identb, identb)

                def win(i, g, dr, ds):
                    """[128, 4, 96] strided window: rows (9+4g+dr).., col 8+ds."""
                    base = (9 + 4 * g + dr) * SP + 8 + ds
                    return xpf[i][:, base:base + 4 * SP] \
                        .rearrange("p (r s) -> p r s", s=SP)[:, :, :96]

                def stage1(i, g):
                    """gate + both expert convs + moe blend for (branch, group)."""
                    plg = psC.tile([HD, GN], F32, tag="ps")
                    nc.tensor.matmul(plg, wgd_sb[i], win(i, g, 0, 0),
                                     start=True, stop=True)
                    ex = gp3.tile([HD, GN], BF16, tag="ex")
                    nc.scalar.activation(ex, plg,
                                         mybir.ActivationFunctionType.Tanh,
                                         scale=-0.5)
                    pa = psC.tile([HD, GN], F32, tag="ps")
                    for ti, (dr, ds) in enumerate(TAPS_A[i]):
                        nc.tensor.matmul(pa, wca_sb[i][:, ti, :], win(i, g, dr, ds),
                                         start=(ti == 0), stop=(ti == 8))
                    pb = psC.tile([HD, GN], F32, tag="ps")
                    for ti, (dr, ds) in enumerate(TAPS_B[i]):
                        nc.tensor.matmul(pb, wcb_sb[i][:, ti, :], win(i, g, dr, ds),
                                         start=(ti == 0), stop=(ti == 8))
                    # moe = g0*(ca - cb) + cb  (bias-add fused on ACT)
                    ca = gp3.tile([HD, GN], BF16, tag="ca")
                    nc.scalar.activation(ca, pa,
                                         mybir.ActivationFunctionType.Identity,
                                         bias=bca_sb[:, i:i + 1], scale=0.5)
                    cb = gp3.tile([HD, GN], BF16, tag="cb")
                    nc.scalar.activation(cb, pb,
                                         mybir.ActivationFunctionType.Identity,
                                         bias=bcb_sb[:, i:i + 1], scale=0.5)
                    dd = gp3.tile([HD, GN], BF16, tag="dd")
                    nc.vector.tensor_sub(dd, ca, cb)
                    d2 = gp3.tile([HD, GN], BF16, tag="d2")
                    nc.vector.tensor_mul(d2, dd, ex)
                    ss = gp3.tile([HD, GN], BF16, tag="ss")
                    nc.vector.tensor_add(ss, ca, cb)
                    moe = gp6.tile([HD, GN], BF16, tag="moe")
                    nc.vector.tensor_add(moe, ss, d2)
                    return moe

                def stage2(i, moe):
                    """fused qh = (scale*Wq@Wk^T)^T moe and v (w/ proj fused)."""
                    pqh = psT.tile([HD, GN], F32, tag="ps")
                    nc.tensor.matmul(pqh, wqh_sb[i], moe, start=True, stop=True)
                    qh = gp6.tile([HD, GN], BF16, tag="qh")
                    nc.scalar.copy(qh, pqh)
                    pvt = psT.tile([96, 4 * HD], F32, tag="ps")
                    for j in range(4):
                        nc.tensor.matmul(pvt[:, j * HD:(j + 1) * HD],
                                         moe[:, j * 96:(j + 1) * 96],
                                         wv_sb[i], start=True, stop=True)
                    vt_sb = ap9.tile([96, 4 * HD], BF16, tag="vt")
                    nc.vector.tensor_copy(vt_sb, pvt)
                    return qh, vt_sb

                def stage3(qh, moe):
                    """scores + softmax numerator/denominator."""
                    psc = psT.tile([96, GN], F32, tag="ps")
                    for j in range(4):
                        nc.tensor.matmul(psc[:, j * 96:(j + 1) * 96],
                                         qh[:, j * 96:(j + 1) * 96],
                                         moe[:, j * 96:(j + 1) * 96],
                                         start=True, stop=True)
                    probs = ap3.tile([96, GN], BF16, tag="probs")
                    nc.scalar.activation(probs, psc,
                                         mybir.ActivationFunctionType.Exp)
                    zsum = ap3.tile([96, 4], F32, tag="zsum")
                    nc.vector.tensor_reduce(
                        zsum, probs.rearrange("p (j q) -> p j q", q=96),
                        axis=mybir.AxisListType.X, op=mybir.AluOpType.add)
                    rec = ap3.tile([96, 4], F32, tag="rec")
                    nc.vector.reciprocal(rec, zsum)
                    pn = gp6.tile([96, GN], BF16, tag="pn")
                    for j in range(4):
                        nc.vector.tensor_scalar_mul(
                            pn[:, j * 96:(j + 1) * 96],
                            probs[:, j * 96:(j + 1) * 96], rec[:, j:j + 1])
                    return pn

                def stage4(i, g, pn, vt_sb):
                    """probs transpose + attention output + xc write."""
                    ppt = psT.tile([96, GN], BF16, tag="ps")
                    for j in range(4):
                        nc.tensor.transpose(ppt[:, j * 96:(j + 1) * 96],
                                            pn[:, j * 96:(j + 1) * 96],
                                            identb[:96, :96])
                    pt_sb = ap3.tile([96, GN], BF16, tag="pt")
                    nc.vector.tensor_copy(pt_sb, ppt)
                    po = psT.tile([HD, GN], F32, tag="ps")
                    for j in range(4):
                        nc.tensor.matmul(po[:, j * 96:(j + 1) * 96],
                                         vt_sb[:, j * HD:(j + 1) * HD],
                                         pt_sb[:, j * 96:(j + 1) * 96],
                                         start=True, stop=True)
                    nc.vector.tensor_scalar(
                        xc_t[i][:, g * GN:(g + 1) * GN], po,
                        bap_sb[:, i:i + 1], None, op0=mybir.AluOpType.add)

                s2 = {}  # g -> [(qh, vt), ...]; smoe: g -> [moe, ...]
                smoe = {}
                s3 = {}  # g -> [pn, ...]
                for g in range(GROUPS):
                    moes = []
                    for i in range(3):
                        moes.append(stage1(i, g))
                        if i == 0 and g - 1 in s2:
                            s3[g - 1] = [stage3(s2[g - 1][ii][0],
                                                smoe[g - 1][ii])
                                         for ii in range(3)]
                        if i == 1 and g - 2 in s3:
                            for ii in range(3):
                                stage4(ii, g - 2, s3[g - 2][ii],
                                       s2[g - 2][ii][1])
                            del s3[g - 2], s2[g - 2], smoe[g - 2]
                    # stream Phase-B weights on the scalar queue mid-phase
                    if 2 <= g <= 6:
                        for dst, src in phase_b_loads[2 * (g - 2):2 * (g - 1)]:
                            nc.scalar.dma_start(out=dst, in_=src)
                    s2[g] = [stage2(i, moes[i]) for i in range(3)]
                    smoe[g] = moes
                g = GROUPS
                s3[g - 1] = [stage3(s2[g - 1][ii][0], smoe[g - 1][ii])
                             for ii in range(3)]
                for gg in (g - 2, g - 1):
                    for ii in range(3):
                        stage4(ii, gg, s3[gg][ii], s2[gg][ii][1])

            # ------- Phase B gating: precomputed for all 4608 tokens -------
            # (emitted right after the Phase-A drain; the small matmuls and
            # the batched token-major top-2 chain fill the drain bubbles)
            NB = T // HD  # 36 column-blocks of 128 tokens
            with tc.tile_pool(name="gateS", bufs=1) as gateS:
                lsb_all = gateS.tile([3, T], F32, name="lsb_all")
                lt_all = gateS.tile([HD, 3 * NB], F32, name="lt_all")
                gates_all = gateS.tile([3, T], BF16, name="gates_all")
                with tc.tile_pool(name="psG", bufs=2, space="PSUM") as psG:
                    for t0, nt in TILES:
                        plg = psG.tile([3, 512], F32, tag="plg", name="plg")
                        for kc in range(3):
                            nc.tensor.matmul(plg[:, :nt], wgf_sb[:, kc, :3],
                                             xc_t[kc][:, t0:t0 + nt],
                                             start=(kc == 0), stop=(kc == 2))
                        nc.vector.tensor_copy(lsb_all[:, t0:t0 + nt],
                                              plg[:, :nt])
                    plt = psG.tile([HD, 3 * NB], F32, tag="plt", name="plt")
                    for b in range(NB):
                        nc.tensor.transpose(plt[:, 3 * b:3 * b + 3],
                                            lsb_all[:, HD * b:HD * (b + 1)],
                                            ident[:3, :3])
                    nc.vector.tensor_copy(lt_all, plt)
                    l3 = lt_all.rearrange("p (b e) -> p b e", e=3)
                    mx = gateS.tile([HD, NB], F32, name="mx")
                    nc.vector.tensor_reduce(mx, l3, axis=mybir.AxisListType.X,
                                            op=mybir.AluOpType.max)
                    mn = gateS.tile([HD, NB], F32, name="mn")
                    nc.vector.tensor_reduce(mn, l3, axis=mybir.AxisListType.X,
                                            op=mybir.AluOpType.min)
                    sm = gateS.tile([HD, NB], F32, name="sm")
                    nc.vector.tensor_reduce(sm, l3, axis=mybir.AxisListType.X,
                                            op=mybir.AluOpType.add)
                    t1 = gateS.tile([HD, NB], F32, name="t1")
                    nc.vector.tensor_sub(t1, sm, mx)
                    mid = gateS.tile([HD, NB], F32, name="mid")
                    nc.vector.tensor_sub(mid, t1, mn)
                    dm = gateS.tile([HD, NB], F32, name="dm")
                    nc.vector.tensor_sub(dm, mx, mid)
                    th = gateS.tile([HD, NB], F32, name="th")
                    nc.scalar.activation(th, dm,
                                         mybir.ActivationFunctionType.Tanh,
                                         scale=0.5)
                    gmx = gateS.tile([HD, NB], F32, name="gmx")
                    nc.vector.tensor_scalar(gmx, th, 0.5, 0.5,
                                            op0=mybir.AluOpType.mult,
                                            op1=mybir.AluOpType.add)
                    eqx = gateS.tile([HD, 3 * NB], F32, name="eqx")
                    eqn = gateS.tile([HD, 3 * NB], F32, name="eqn")
                    e3 = lambda tl: tl.rearrange("p (b e) -> p b e", e=3)
                    bc = lambda tl: tl[:, :, None].to_broadcast([HD, NB, 3])
                    nc.vector.tensor_tensor(e3(eqx), l3, bc(mx),
                                            op=mybir.AluOpType.is_equal)
                    nc.vector.tensor_tensor(e3(eqn), l3, bc(mn),
                                            op=mybir.AluOpType.is_equal)
                    # u = 1 - eqx - eqn (mid indicator); g = gmx*(eqx-u) + u
                    s1 = gateS.tile([HD, 3 * NB], F32, name="s1")
                    nc.vector.tensor_add(s1, eqx, eqn)
                    u = gateS.tile([HD, 3 * NB], F32, name="u")
                    nc.vector.tensor_scalar(u, s1, -1.0, 1.0,
                                            op0=mybir.AluOpType.mult,
                                            op1=mybir.AluOpType.add)
                    d0 = gateS.tile([HD, 3 * NB], F32, name="d0")
                    nc.vector.tensor_sub(d0, eqx, u)
                    p0 = gateS.tile([HD, 3 * NB], F32, name="p0")
                    nc.vector.tensor_tensor(e3(p0), e3(d0), bc(gmx),
                                            op=mybir.AluOpType.mult)
                    gm_all = gateS.tile([HD, 3 * NB], BF16, name="gm_all")
                    nc.vector.tensor_add(gm_all, p0, u)
                    for c0 in range(0, NB, 4):  # expert-major gates per 512
                        pgt = psG.tile([3, 512], BF16, tag="pgt", name="pgt")
                        for t4 in range(4):
                            nc.tensor.transpose(
                                pgt[:, t4 * HD:(t4 + 1) * HD],
                                gm_all[:, 3 * (c0 + t4):3 * (c0 + t4) + 3],
                                identb)
                        nc.scalar.copy(gates_all[:, c0 * HD:(c0 + 4) * HD],
                                       pgt)

                # ------------ Phase B: final MLP MoE + proj ---------------
                with tc.tile_pool(name="bpool", bufs=3) as bpool, \
                     tc.tile_pool(name="psL", bufs=3, space="PSUM") as psL, \
                     tc.tile_pool(name="psPG", bufs=1, space="PSUM") as psPG, \
                     tc.tile_pool(name="psB", bufs=4, space="PSUM") as psB:
                    for t0, nt in TILES:
                        gts = gates_all[:, t0:t0 + nt]
                        pd = [psL.tile([HD, 512], F32, tag="down", name=f"pd{_i}") for _i in range(3)]
                        for e in range(3):
                            pgb = psPG.tile([HD, 512], F32, tag="pgb", name="pgb")
                            nc.tensor.matmul(pgb[:, :nt],
                                             eb3_sb[:, e * HD:(e + 1) * HD],
                                             gts, start=True, stop=True)
                            for m in range(12):
                                pu = psB.tile([HD, 512], F32, tag="ps", name="pu")
                                for kc in range(3):
                                    nc.tensor.matmul(
                                        pu[:, :nt],
                                        w1_sb[e][:, kc, m * HD:(m + 1) * HD],
                                        xc_t[kc][:, t0:t0 + nt],
                                        start=(kc == 0), stop=(kc == 2))
                                h = bpool.tile([HD, 512], F32, tag="h")
                                nc.scalar.activation(
                                    h[:, :nt], pu[:, :nt],
                                    mybir.ActivationFunctionType.Gelu,
                                    bias=b1_sb[:, e, m:m + 1])
                                hs = bpool.tile([HD, 512], BF16, tag="hs")
                                nc.vector.tensor_mul(hs[:, :nt], h[:, :nt],
                                                     pgb[:, :nt])
                                for mp in range(3):
                                    nc.tensor.matmul(
                                        pd[mp][:, :nt],
                                        w2_sb[e][:, m, mp * HD:(mp + 1) * HD],
                                        hs[:, :nt], start=(e == 0 and m == 0),
                                        stop=False)
                        for mp in range(3):
                            nc.tensor.matmul(pd[mp][:, :nt],
                                             b2r_sb[:, mp * HD:(mp + 1) * HD],
                                             gts, start=False, stop=True)
                        for mp in range(3):
                            osb = bpool.tile([HD, 512], F32, tag="osb")
                            nc.scalar.activation(
                                osb[:, :nt], pd[mp][:, :nt],
                                mybir.ActivationFunctionType.Identity,
                                bias=bpr_sb[:, mp:mp + 1])
                            nc.sync.dma_start(
                                out=out_cm[mp * HD:(mp + 1) * HD, t0:t0 + nt],
                                in_=osb[:, :nt])
    nc.compile()
    return nc


def _prep_inputs(x, w_e1, b_e1, w_e2, b_e2, w_e3, b_e3, w_e4, b_e4, w_e5, b_e5,
                 w_e6, b_e6, wg1, wg2, wg3, w_qkv, w_attn_proj, b_attn_proj,
                 wg_final, w_mlp1, b_mlp1, w_mlp2, b_mlp2, w_proj, b_proj):
    f = np.float32
    shared = {}
    # conv weights pre-transposed to [cin(p), tap, cout] for contiguous DMA
    shared["wca"] = np.ascontiguousarray(np.stack([
        w_e1.reshape(9, HD, HD).transpose(1, 0, 2),
        w_e3.reshape(9, HD, HD).transpose(1, 0, 2),
        w_e5.reshape(9, HD, HD).transpose(1, 0, 2)]).astype(BF))
    shared["wcb"] = np.ascontiguousarray(np.stack([
        w_e2.reshape(9, HD, HD).transpose(1, 0, 2),
        w_e4.reshape(9, HD, HD).transpose(1, 0, 2),
        w_e6.reshape(9, HD, HD).transpose(1, 0, 2)]).astype(BF))
    shared["bca"] = np.ascontiguousarray(
        np.stack([b_e1, b_e3, b_e5], axis=1) * 0.5, dtype=f)
    shared["bcb"] = np.ascontiguousarray(
        np.stack([b_e2, b_e4, b_e6], axis=1) * 0.5, dtype=f)
    wgs = np.stack([wg1, wg2, wg3])
    shared["wgd"] = np.ascontiguousarray(
        np.repeat((wgs[:, :, 1] - wgs[:, :, 0])[:, :, None], HD, axis=2)
        .astype(BF))
    eb3 = np.zeros((3, 384), f)
    for e in range(3):
        eb3[e, e * 128:(e + 1) * 128] = 1.0
    shared["eb3"] = eb3.astype(BF)
    # fused score matrix A = SCALE * Wq @ Wk^T  (scores = moe A moe^T)
    wq64 = np.asarray(w_qkv[:, :, :HD], dtype=np.float64)
    wk64 = np.asarray(w_qkv[:, :, HD:2 * HD], dtype=np.float64)
    shared["wqh"] = np.ascontiguousarray(
        (SCALE * np.einsum("iac,ibc->iab", wq64, wk64)).astype(BF))
    wv64 = np.asarray(w_qkv[:, :, 2 * HD:], dtype=np.float64)
    wap64 = np.asarray(w_attn_proj, dtype=np.float64)
    shared["wv"] = np.ascontiguousarray(
        np.einsum("ick,iko->ico", wv64, wap64).astype(BF))
    shared["bap"] = np.ascontiguousarray(b_attn_proj.T, dtype=f)
    shared["wgf"] = np.ascontiguousarray(
        np.tile(wg_final.reshape(3, HD, 3), (1, 1, 43))[:, :, :HD].astype(BF))
    shared["w1"] = np.ascontiguousarray(
        w_mlp1.reshape(3, 3, HD, 1536).transpose(0, 2, 1, 3).astype(BF))
    shared["b1"] = np.ascontiguousarray(
        b_mlp1.reshape(3, 12, HD).transpose(2, 0, 1), dtype=f)
    w2p = np.asarray(w_mlp2, dtype=np.float64) @ np.asarray(w_proj, np.float64)
    shared["w2"] = np.ascontiguousarray(
        w2p.reshape(3, 12, HD, C).transpose(0, 2, 1, 3).astype(BF))
    shared["b2r"] = np.ascontiguousarray(
        (np.asarray(b_mlp2, np.float64) @ np.asarray(w_proj, np.float64))
        .astype(BF))
    shared["bpr"] = np.ascontiguousarray(b_proj.reshape(3, HD).T, dtype=f)

    in_maps = []
    for c in range(N_CORES):
        b, half = c // 2, c % 2
        r0 = half * R
        slab = np.zeros((C, RP, SP), BF)
        glo, ghi = max(0, r0 - 8), min(HH, r0 + R + 8)
        plo = glo - (r0 - 8) + 1
        slab[:, plo:plo + (ghi - glo), 8:SP] = \
            np.asarray(x[b, glo:ghi]).astype(BF).transpose(2, 0, 1)
        m = dict(shared)
        m["xp"] = np.ascontiguousarray(slab)
        in_maps.append(m)
    return in_maps


def kernel(**inputs):
    global _CACHED_NC
    if _CACHED_NC is None:
        _CACHED_NC = build_kernel()
    nc = _CACHED_NC
    in_maps = _prep_inputs(**{k: np.asarray(v) for k, v in inputs.items()})
    res = None
    for attempt in range(3):
        try:
            res = run_bass_kernel_spmd(nc, in_maps,
                                       core_ids=list(range(N_CORES)))
            break
        except Exception:
            if attempt == 2:
                raise
            import time
            time.sleep(2.0)
    out = np.empty((B, HH, WW, C), np.float32)
    for c in range(N_CORES):
        b, half = c // 2, c % 2
        slab = res.results[c]["out_cm"].reshape(C, R, 96)
        out[b, :, half * R:(half + 1) * R, :] = slab.transpose(2, 1, 0)
    return out


# revision 9
# speedup vs baseline: 1.0109x; 1.0109x over previous
"""Trainium2 Bass kernel for nn_MAMoE (conv-MoE -> row attention -> MLP-MoE).

Sharding: 8 cores = (batch b in 0..3) x (H-half in 0..1). All routing is
per-token; the reference's swapaxes(1,2) means attention row r produces
output column w=r, so each core independently computes the full pipeline
for its 48 attention rows and the host reassembles along W.

Layout: padded row stride 104 (8 zero cols serve as both right halo of
row r and left halo of row r+1); conv/gate matmuls use strided rhs APs
([4 rows @ 104, 96]) so no pad columns are ever computed. scores use a
host-fused A = scale * Wq @ Wk^T so only one projection (qh) is needed.
bf16 everywhere with fp32 PSUM accumulation. Phase A is branch-interleaved
and software-pipelined two groups deep (scores of group g-1 and attention
tail of group g-2 are emitted under group g's convs) so the in-order PE
queue never blocks on the ACT/DVE softmax chain. Phase-B weights preload
on the second hardware DMA queue during Phase A.
"""
import numpy as np
import ml_dtypes

import concourse.bass as bass
import concourse.mybir as mybir
import concourse.tile as tile
from concourse import bacc
from concourse.bass_utils import run_bass_kernel_spmd
from concourse.masks import make_identity

F32 = mybir.dt.float32
F32R = mybir.dt.float32r
BF16 = mybir.dt.bfloat16
BF = ml_dtypes.bfloat16

B, HH, WW, C = 4, 96, 96, 384
HD = 128
SCALE = float((HD // 3) ** -0.5)  # 42**-0.5
N_CORES = 8
R = 48            # attention rows per core
RP = 66           # slack row + 8 halo + 48 + 8 halo + 1 slack row
SP = 104          # padded row stride (8 zero pad + 96 valid)
T = R * 96        # tokens per core = 4608
GROUPS = R // 4   # 12 groups of 4 rows
GN = 4 * 96       # tokens per group = 384
# MLP tiles: 8x512 then 2x256 (narrow tail shortens the end-of-kernel drain)
TILES = [(t * 512, 512) for t in range(8)] + [(4096, 256), (4352, 256)]

TAPS_A = [
    [(dr, ds) for dr in (-1, 0, 1) for ds in (-1, 0, 1)],
    [(dr, 0) for dr in range(-4, 5)],
    [(0, ds) for ds in range(-4, 5)],
]
TAPS_B = [
    [(dr, ds) for dr in (-2, 0, 2) for ds in (-2, 0, 2)],
    [(dr, 0) for dr in range(-8, 9, 2)],
    [(0, ds) for ds in range(-8, 9, 2)],
]

_CACHED_NC = None


def build_kernel():
    nc = bacc.Bacc("TRN2", target_bir_lowering=False, debug=False)

    xp = nc.dram_tensor("xp", [C, RP, SP], BF16, kind="ExternalInput").ap()
    wca = nc.dram_tensor("wca", [3, HD, 9, HD], BF16, kind="ExternalInput").ap()
    wcb = nc.dram_tensor("wcb", [3, HD, 9, HD], BF16, kind="ExternalInput").ap()
    bca = nc.dram_tensor("bca", [HD, 3], F32, kind="ExternalInput").ap()
    bcb = nc.dram_tensor("bcb", [HD, 3], F32, kind="ExternalInput").ap()
    wgd = nc.dram_tensor("wgd", [3, HD, HD], BF16, kind="ExternalInput").ap()
    eb3 = nc.dram_tensor("eb3", [3, 384], BF16, kind="ExternalInput").ap()
    wqh = nc.dram_tensor("wqh", [3, HD, HD], BF16, kind="ExternalInput").ap()
    wv = nc.dram_tensor("wv", [3, HD, HD], BF16, kind="ExternalInput").ap()
    bap = nc.dram_tensor("bap", [HD, 3], F32, kind="ExternalInput").ap()
    wgf = nc.dram_tensor("wgf", [3, HD, HD], BF16, kind="ExternalInput").ap()
    w1 = nc.dram_tensor("w1", [3, HD, 3, 1536], BF16, kind="ExternalInput").ap()
    b1 = nc.dram_tensor("b1", [HD, 3, 12], F32, kind="ExternalInput").ap()
    w2 = nc.dram_tensor("w2", [3, HD, 12, C], BF16, kind="ExternalInput").ap()
    b2r = nc.dram_tensor("b2r", [3, C], BF16, kind="ExternalInput").ap()
    bpr = nc.dram_tensor("bpr", [HD, 3], F32, kind="ExternalInput").ap()
    out_cm = nc.dram_tensor("out_cm", [C, T], F32, kind="ExternalOutput").ap()

    with tile.TileContext(nc) as tc:
        with tc.tile_pool(name="consts", bufs=1) as consts, \
             tc.tile_pool(name="persist", bufs=1) as persist:
            ident = consts.tile([HD, HD], F32)
            make_identity(nc, ident)
            identb = consts.tile([HD, HD], BF16)
            nc.vector.tensor_copy(identb, ident)

            bca_sb = persist.tile([HD, 3], F32)
            bcb_sb = persist.tile([HD, 3], F32)
            bap_sb = persist.tile([HD, 3], F32)

            xc_t = [persist.tile([HD, T], BF16, tag=f"xc{i}", name=f"xc{i}") for i in range(3)]

            # Phase-B weights (DMAs issued later, on the scalar HWDGE queue)
            b1_sb = persist.tile([HD, 3, 12], F32)
            b2r_sb = persist.tile([3, C], BF16)
            wgf_sb = persist.tile([HD, 3, HD], BF16)
            bpr_sb = persist.tile([HD, 3], F32)
            eb3_sb = persist.tile([3, 384], BF16)
            w1_sb = [persist.tile([HD, 3, 1536], BF16, tag=f"w1_{e}", name=f"w1_{e}")
                     for e in range(3)]
            w2_sb = [persist.tile([HD, 12, C], BF16, tag=f"w2_{e}", name=f"w2_{e}")
                     for e in range(3)]

            phase_b_loads = []
            for e in range(3):
                phase_b_loads.append((w1_sb[e], w1[e]))
                phase_b_loads.append((w2_sb[e], w2[e]))
            phase_b_loads += [
                (b1_sb, b1), (b2r_sb, b2r), (bpr_sb, bpr), (eb3_sb, eb3),
            ]

            # ---------------- Phase A: conv MoE + attention, interleaved --
            with tc.tile_pool(name="xw", bufs=1) as xw, \
                 tc.tile_pool(name="gp3", bufs=3) as gp3, \
                 tc.tile_pool(name="gp6", bufs=6) as gp6, \
                 tc.tile_pool(name="ap3", bufs=3) as ap3, \
                 tc.tile_pool(name="ap9", bufs=9) as ap9, \
                 tc.tile_pool(name="psC", bufs=3, space="PSUM") as psC, \
                 tc.tile_pool(name="psT", bufs=5, space="PSUM") as psT:
                xp_sb = [xw.tile([HD, RP, SP], BF16, tag=f"xp{i}", name=f"xp{i}")
                         for i in range(3)]
                wgd_sb, wca_sb, wcb_sb, wqh_sb, wv_sb = [], [], [], [], []
                for i in range(3):
                    wgd_sb.append(xw.tile([HD, HD], BF16, tag=f"wgd{i}", name=f"wgd{i}"))
                    wca_sb.append(xw.tile([HD, 9, HD], BF16, tag=f"wca{i}", name=f"wca{i}"))
                    wcb_sb.append(xw.tile([HD, 9, HD], BF16, tag=f"wcb{i}", name=f"wcb{i}"))
                    wqh_sb.append(xw.tile([HD, HD], BF16, tag=f"wqh{i}", name=f"wqh{i}"))
                    wv_sb.append(xw.tile([HD, HD], BF16, tag=f"wv{i}", name=f"wv{i}"))

                def loadw(i, eng):
                    eng.dma_start(out=wgd_sb[i], in_=wgd[i])
                    eng.dma_start(out=wca_sb[i], in_=wca[i])
                    eng.dma_start(out=wcb_sb[i], in_=wcb[i])
                    eng.dma_start(out=wqh_sb[i], in_=wqh[i])
                    eng.dma_start(out=wv_sb[i], in_=wv[i])

                # Criticality-ordered DMA issue across both HWDGE queues.
                nc.sync.dma_start(out=xp_sb[0][:, :13, :], in_=xp[0:HD, :13, :])
                nc.scalar.dma_start(out=xp_sb[0][:, 13:25, :],
                                    in_=xp[0:HD, 13:25, :])
                loadw(0, nc.sync)
                nc.scalar.dma_start(out=xp_sb[1][:, :25, :],
                                    in_=xp[HD:2 * HD, :25, :])
                nc.scalar.dma_start(out=xp_sb[2][:, :25, :],
                                    in_=xp[2 * HD:3 * HD, :25, :])
                nc.scalar.dma_start(out=bca_sb, in_=bca)
                nc.scalar.dma_start(out=bcb_sb, in_=bcb)
                nc.scalar.dma_start(out=bap_sb, in_=bap)
                loadw(1, nc.sync)
                loadw(2, nc.sync)
                for i in range(3):
                    nc.sync.dma_start(out=xp_sb[i][:, 25:45, :],
                                      in_=xp[i * HD:(i + 1) * HD, 25:45, :])
                for i in range(3):
                    nc.sync.dma_start(out=xp_sb[i][:, 45:, :],
                                      in_=xp[i * HD:(i + 1) * HD, 45:, :])
                nc.sync.dma_start(out=wgf_sb, in_=wgf.rearrange("a p b -> p a b"))

                xpf = [xp_sb[i].rearrange("p r s -> p (r s)") for i in range(3)]

                def win(i, g, dr, ds):
                    """[128, 4, 96] strided window: rows (9+4g+dr).., col 8+ds."""
                    base = (9 + 4 * g + dr) * SP + 8 + ds
                    return xpf[i][:, base:base + 4 * SP] \
                        .rearrange("p (r s) -> p r s", s=SP)[:, :, :96]

                def stage1(i, g):
                    """gate + both expert convs + moe blend for (branch, group)."""
                    plg = psC.tile([HD, GN], F32, tag="ps")
                    nc.tensor.matmul(plg, wgd_sb[i], win(i, g, 0, 0),
                                     start=True, stop=True)
                    ex = gp3.tile([HD, GN], BF16, tag="ex")
                    nc.scalar.activation(ex, plg,
                                         mybir.ActivationFunctionType.Tanh,
                                         scale=-0.5)
                    pa = psC.tile([HD, GN], F32, tag="ps")
                    for ti, (dr, ds) in enumerate(TAPS_A[i]):
                        nc.tensor.matmul(pa, wca_sb[i][:, ti, :], win(i, g, dr, ds),
                                         start=(ti == 0), stop=(ti == 8))
                    pb = psC.tile([HD, GN], F32, tag="ps")
                    for ti, (dr, ds) in enumerate(TAPS_B[i]):
                        nc.tensor.matmul(pb, wcb_sb[i][:, ti, :], win(i, g, dr, ds),
                                         start=(ti == 0), stop=(ti == 8))
                    # moe = g0*(ca - cb) + cb  (bias-add fused on ACT)
                    ca = gp3.tile([HD, GN], BF16, tag="ca")
                    nc.scalar.activation(ca, pa,
                                         mybir.ActivationFunctionType.Identity,
                                         bias=bca_sb[:, i:i + 1], scale=0.5)
                    cb = gp3.tile([HD, GN], BF16, tag="cb")
                    nc.scalar.activation(cb, pb,
                                         mybir.ActivationFunctionType.Identity,
                                         bias=bcb_sb[:, i:i + 1], scale=0.5)
                    dd = gp3.tile([HD, GN], BF16, tag="dd")
                    nc.vector.tensor_sub(dd, ca, cb)
                    d2 = gp3.tile([HD, GN], BF16, tag="d2")
                    nc.vector.tensor_mul(d2, dd, ex)
                    ss = gp3.tile([HD, GN], BF16, tag="ss")
                    nc.vector.tensor_add(ss, ca, cb)
                    moe = gp6.tile([HD, GN], BF16, tag="moe")
                    nc.vector.tensor_add(moe, ss, d2)
                    return moe

                def stage2(i, moe):
                    """fused qh = (scale*Wq@Wk^T)^T moe and v (w/ proj fused)."""
                    pqh = psT.tile([HD, GN], F32, tag="ps")
                    nc.tensor.matmul(pqh, wqh_sb[i], moe, start=True, stop=True)
                    qh = gp6.tile([HD, GN], BF16, tag="qh")
                    nc.scalar.copy(qh, pqh)
                    pvt = psT.tile([96, 4 * HD], F32, tag="ps")
                    for j in range(4):
                        nc.tensor.matmul(pvt[:, j * HD:(j + 1) * HD],
                                         moe[:, j * 96:(j + 1) * 96],
                                         wv_sb[i], start=True, stop=True)
                    vt_sb = ap9.tile([96, 4 * HD], BF16, tag="vt")
                    nc.vector.tensor_copy(vt_sb, pvt)
                    return qh, vt_sb

                def stage3(qh, moe):
                    """scores + softmax numerator/denominator."""
                    psc = psT.tile([96, GN], F32, tag="ps")
                    for j in range(4):
                        nc.tensor.matmul(psc[:, j * 96:(j + 1) * 96],
                                         qh[:, j * 96:(j + 1) * 96],
                                         moe[:, j * 96:(j + 1) * 96],
                                         start=True, stop=True)
                    probs = ap3.tile([96, GN], BF16, tag="probs")
                    nc.scalar.activation(probs, psc,
                                         mybir.ActivationFunctionType.Exp)
                    zsum = ap3.tile([96, 4], F32, tag="zsum")
                    nc.vector.tensor_reduce(
                        zsum, probs.rearrange("p (j q) -> p j q", q=96),
                        axis=mybir.AxisListType.X, op=mybir.AluOpType.add)
                    rec = ap3.tile([96, 4], F32, tag="rec")
                    nc.vector.reciprocal(rec, zsum)
                    pn = gp6.tile([96, GN], BF16, tag="pn")
                    for j in range(4):
                        nc.vector.tensor_scalar_mul(
                            pn[:, j * 96:(j + 1) * 96],
                            probs[:, j * 96:(j + 1) * 96], rec[:, j:j + 1])
                    return pn

                def stage4(i, g, pn, vt_sb):
                    """probs transpose + attention output + xc write."""
                    ppt = psT.tile([96, GN], BF16, tag="ps")
                    for j in range(4):
                        nc.tensor.transpose(ppt[:, j * 96:(j + 1) * 96],
                                            pn[:, j * 96:(j + 1) * 96],
                                            identb[:96, :96])
                    pt_sb = ap3.tile([96, GN], BF16, tag="pt")
                    nc.vector.tensor_copy(pt_sb, ppt)
                    po = psT.tile([HD, GN], F32, tag="ps")
                    for j in range(4):
                        nc.tensor.matmul(po[:, j * 96:(j + 1) * 96],
                                         vt_sb[:, j * HD:(j + 1) * HD],
                                         pt_sb[:, j * 96:(j + 1) * 96],
                                         start=True, stop=True)
                    nc.vector.tensor_scalar(
                        xc_t[i][:, g * GN:(g + 1) * GN], po,
                        bap_sb[:, i:i + 1], None, op0=mybir.AluOpType.add)

                s2 = {}  # g -> [(qh, vt), ...]; smoe: g -> [moe, ...]
                smoe = {}
                s3 = {}  # g -> [pn, ...]
                for g in range(GROUPS):
                    moes = []
                    for i in range(3):
                        moes.append(stage1(i, g))
                        if i == 0 and g - 1 in s2:
                            s3[g - 1] = [stage3(s2[g - 1][ii][0],
                                                smoe[g - 1][ii])
                                         for ii in range(3)]
                        if i == 1 and g - 2 in s3:
                            for ii in range(3):
                                stage4(ii, g - 2, s3[g - 2][ii],
                                       s2[g - 2][ii][1])
                            del s3[g - 2], s2[g - 2], smoe[g - 2]
                    # stream Phase-B weights on the scalar queue mid-phase
                    if 2 <= g <= 6:
                        for dst, src in phase_b_loads[2 * (g - 2):2 * (g - 1)]:
                            nc.scalar.dma_start(out=dst, in_=src)
                    s2[g] = [stage2(i, moes[i]) for i in range(3)]
                    smoe[g] = moes
                g = GROUPS
                s3[g - 1] = [stage3(s2[g - 1][ii][0], smoe[g - 1][ii])
                             for ii in range(3)]
                for gg in (g - 2, g - 1):
                    for ii in range(3):
                        stage4(ii, gg, s3[gg][ii], s2[gg][ii][1])

            # ---------------- Phase B: final MLP MoE + proj ---------------
            with tc.tile_pool(name="bpool", bufs=3) as bpool, \
                 tc.tile_pool(name="gpoolB", bufs=2) as gpoolB, \
                 tc.tile_pool(name="psL", bufs=3, space="PSUM") as psL, \
                 tc.tile_pool(name="psGB", bufs=1, space="PSUM") as psGB, \
                 tc.tile_pool(name="psPG", bufs=1, space="PSUM") as psPG, \
                 tc.tile_pool(name="psB", bufs=3, space="PSUM") as psB:

                def gating_part1a(t):
                    """logits matmul for tile t."""
                    t0, nt = TILES[t]
                    plg = psGB.tile([3, 512], F32, tag="ps", name="plg")
                    for kc in range(3):
                        nc.tensor.matmul(plg[:, :nt], wgf_sb[:, kc, :3],
                                         xc_t[kc][:, t0:t0 + nt],
                                         start=(kc == 0), stop=(kc == 2))
                    lsb = gpoolB.tile([3, 512], F32, tag="lsb", name="lsb")
                    nc.vector.tensor_copy(lsb[:, :nt], plg[:, :nt])
                    return lsb

                def gating_part1b(lsb, t):
                    """token-major top-2 softmax math."""
                    nt = TILES[t][1]
                    n4 = nt // HD  # 4 or 2 column-blocks of 128 tokens
                    plt = psGB.tile([HD, 12], F32, tag="ps", name="plt")
                    for t4 in range(n4):
                        nc.tensor.transpose(plt[:, t4 * 3:(t4 + 1) * 3],
                                            lsb[:, t4 * HD:(t4 + 1) * HD],
                                            ident[:3, :3])
                    lt = gpoolB.tile([HD, 12], F32, tag="lt", name="lt")
                    nc.vector.tensor_copy(lt[:, :3 * n4], plt[:, :3 * n4])
                    l3 = lt.rearrange("p (j e) -> p j e", e=3)
                    mx = gpoolB.tile([HD, 4], F32, tag="mx", name="mx")
                    nc.vector.tensor_reduce(mx[:, :n4], l3[:, :n4],
                                            axis=mybir.AxisListType.X,
                                            op=mybir.AluOpType.max)
                    mn = gpoolB.tile([HD, 4], F32, tag="mn", name="mn")
                    nc.vector.tensor_reduce(mn[:, :n4], l3[:, :n4],
                                            axis=mybir.AxisListType.X,
                                            op=mybir.AluOpType.min)
                    sm = gpoolB.tile([HD, 4], F32, tag="sm", name="sm")
                    nc.vector.tensor_reduce(sm[:, :n4], l3[:, :n4],
                                            axis=mybir.AxisListType.X,
                                            op=mybir.AluOpType.add)
                    t1 = gpoolB.tile([HD, 4], F32, tag="t1", name="t1")
                    nc.vector.tensor_sub(t1[:, :n4], sm[:, :n4], mx[:, :n4])
                    mid = gpoolB.tile([HD, 4], F32, tag="mid", name="mid")
                    nc.vector.tensor_sub(mid[:, :n4], t1[:, :n4], mn[:, :n4])
                    dm = gpoolB.tile([HD, 4], F32, tag="dm", name="dm")
                    nc.vector.tensor_sub(dm[:, :n4], mx[:, :n4], mid[:, :n4])
                    th = gpoolB.tile([HD, 4], F32, tag="th", name="th")
                    nc.scalar.activation(th[:, :n4], dm[:, :n4],
                                         mybir.ActivationFunctionType.Tanh,
                                         scale=0.5)
                    gmx = gpoolB.tile([HD, 4], F32, tag="gmx", name="gmx")
                    nc.vector.tensor_scalar(gmx[:, :n4], th[:, :n4], 0.5, 0.5,
                                            op0=mybir.AluOpType.mult,
                                            op1=mybir.AluOpType.add)
                    eqx = gpoolB.tile([HD, 12], F32, tag="eqx", name="eqx")
                    eqn = gpoolB.tile([HD, 12], F32, tag="eqn", name="eqn")
                    for t4 in range(n4):
                        sl = slice(t4 * 3, (t4 + 1) * 3)
                        nc.vector.tensor_scalar(eqx[:, sl], lt[:, sl],
                                                mx[:, t4:t4 + 1], None,
                                                op0=mybir.AluOpType.is_equal)
                        nc.vector.tensor_scalar(eqn[:, sl], lt[:, sl],
                                                mn[:, t4:t4 + 1], None,
                                                op0=mybir.AluOpType.is_equal)
                    # u = 1 - eqx - eqn (mid indicator); g = gmx*(eqx-u) + u
                    s1 = gpoolB.tile([HD, 12], F32, tag="s1", name="s1")
                    nc.vector.tensor_add(s1[:, :3 * n4], eqx[:, :3 * n4],
                                         eqn[:, :3 * n4])
                    u = gpoolB.tile([HD, 12], F32, tag="u", name="u")
                    nc.vector.tensor_scalar(u[:, :3 * n4], s1[:, :3 * n4],
                                            -1.0, 1.0,
                                            op0=mybir.AluOpType.mult,
                                            op1=mybir.AluOpType.add)
                    d0 = gpoolB.tile([HD, 12], F32, tag="d0", name="d0")
                    nc.vector.tensor_sub(d0[:, :3 * n4], eqx[:, :3 * n4],
                                         u[:, :3 * n4])
                    p0 = gpoolB.tile([HD, 12], F32, tag="p0", name="p0")
                    for t4 in range(n4):
                        sl = slice(t4 * 3, (t4 + 1) * 3)
                        nc.vector.tensor_scalar_mul(p0[:, sl], d0[:, sl],
                                                    gmx[:, t4:t4 + 1])
                    gm = gpoolB.tile([HD, 12], BF16, tag="gm", name="gm")
                    nc.vector.tensor_add(gm[:, :3 * n4], p0[:, :3 * n4],
                                         u[:, :3 * n4])
                    return gm

                def gating_part2(gm, t):
                    """expert-major gates [3, nt] from token-major gm."""
                    nt = TILES[t][1]
                    n4 = nt // HD
                    pgt = psGB.tile([3, 512], BF16, tag="ps", name="pgt")
                    for t4 in range(n4):
                        nc.tensor.transpose(pgt[:, t4 * HD:(t4 + 1) * HD],
                                            gm[:, t4 * 3:(t4 + 1) * 3],
                                            identb)
                    gates_r = gpoolB.tile([3, 512], BF16, tag="gates",
                                          name="gates_r")
                    nc.scalar.copy(gates_r[:, :nt], pgt[:, :nt])
                    return gates_r

                ntl = len(TILES)
                gates_cur = gating_part2(
                    gating_part1b(gating_part1a(0), 0), 0)
                lsb_next = gating_part1a(1)
                gm_next = None
                gates_next = None
                for t in range(ntl):
                    t0, nt = TILES[t]
                    gts = gates_cur[:, :nt]
                    pd = [psL.tile([HD, 512], F32, tag="down", name=f"pd{_i}") for _i in range(3)]
                    for mp in range(3):
                        nc.tensor.matmul(pd[mp][:, :nt],
                                         b2r_sb[:, mp * HD:(mp + 1) * HD],
                                         gts, start=True, stop=False)
                    for e in range(3):
                        if e == 1 and t + 1 < ntl:
                            gm_next = gating_part1b(lsb_next, t + 1)
                        if e == 2 and t + 1 < ntl:
                            gates_next = gating_part2(gm_next, t + 1)
                            lsb_next = gating_part1a(t + 2) if t + 2 < ntl \
                                else None
                        pgb = psPG.tile([HD, 512], F32, tag="pgb", name="pgb")
                        for m in range(12):
                            pu = psB.tile([HD, 512], F32, tag="ps", name="pu")
                            for kc in range(3):
                                nc.tensor.matmul(
                                    pu[:, :nt],
                                    w1_sb[e][:, kc, m * HD:(m + 1) * HD],
                                    xc_t[kc][:, t0:t0 + nt],
                                    start=(kc == 0), stop=(kc == 2))
                            if m == 0:
                                nc.tensor.matmul(
                                    pgb[:, :nt],
                                    eb3_sb[:, e * HD:(e + 1) * HD],
                                    gts, start=True, stop=True)
                            h = bpool.tile([HD, 512], F32, tag="h")
                            nc.scalar.activation(
                                h[:, :nt], pu[:, :nt],
                                mybir.ActivationFunctionType.Gelu,
                                bias=b1_sb[:, e, m:m + 1])
                            hs = bpool.tile([HD, 512], BF16, tag="hs")
                            nc.vector.tensor_mul(hs[:, :nt], h[:, :nt],
                                                 pgb[:, :nt])
                            for mp in range(3):
                                nc.tensor.matmul(
                                    pd[mp][:, :nt],
                                    w2_sb[e][:, m, mp * HD:(mp + 1) * HD],
                                    hs[:, :nt], start=False,
                                    stop=(e == 2 and m == 11))
                    gates_cur = gates_next
                    for mp in range(3):
                        osb = bpool.tile([HD, 512], F32, tag="osb")
                        nc.scalar.activation(osb[:, :nt], pd[mp][:, :nt],
                                             mybir.ActivationFunctionType.Identity,
                                             bias=bpr_sb[:, mp:mp + 1])
                        nc.sync.dma_start(
                            out=out_cm[mp * HD:(mp + 1) * HD, t0:t0 + nt],
                            in_=osb[:, :nt])
    nc.compile()
    return nc


def _prep_inputs(x, w_e1, b_e1, w_e2, b_e2, w_e3, b_e3, w_e4, b_e4, w_e5, b_e5,
                 w_e6, b_e6, wg1, wg2, wg3, w_qkv, w_attn_proj, b_attn_proj,
                 wg_final, w_mlp1, b_mlp1, w_mlp2, b_mlp2, w_proj, b_proj):
    f = np.float32
    shared = {}
    # conv weights pre-transposed to [cin(p), tap, cout] for contiguous DMA
    shared["wca"] = np.ascontiguousarray(np.stack([
        w_e1.reshape(9, HD, HD).transpose(1, 0, 2),
        w_e3.reshape(9, HD, HD).transpose(1, 0, 2),
        w_e5.reshape(9, HD, HD).transpose(1, 0, 2)]).astype(BF))
    shared["wcb"] = np.ascontiguousarray(np.stack([
        w_e2.reshape(9, HD, HD).transpose(1, 0, 2),
        w_e4.reshape(9, HD, HD).transpose(1, 0, 2),
        w_e6.reshape(9, HD, HD).transpose(1, 0, 2)]).astype(BF))
    shared["bca"] = np.ascontiguousarray(
        np.stack([b_e1, b_e3, b_e5], axis=1) * 0.5, dtype=f)
    shared["bcb"] = np.ascontiguousarray(
        np.stack([b_e2, b_e4, b_e6], axis=1) * 0.5, dtype=f)
    wgs = np.stack([wg1, wg2, wg3])
    shared["wgd"] = np.ascontiguousarray(
        np.repeat((wgs[:, :, 1] - wgs[:, :, 0])[:, :, None], HD, axis=2)
        .astype(BF))
    eb3 = np.zeros((3, 384), f)
    for e in range(3):
        eb3[e, e * 128:(e + 1) * 128] = 1.0
    shared["eb3"] = eb3.astype(BF)
    # fused score matrix A = SCALE * Wq @ Wk^T  (scores = moe A moe^T)
    wq64 = np.asarray(w_qkv[:, :, :HD], dtype=np.float64)
    wk64 = np.asarray(w_qkv[:, :, HD:2 * HD], dtype=np.float64)
    shared["wqh"] = np.ascontiguousarray(
        (SCALE * np.einsum("iac,ibc->iab", wq64, wk64)).astype(BF))
    wv64 = np.asarray(w_qkv[:, :, 2 * HD:], dtype=np.float64)
    wap64 = np.asarray(w_attn_proj, dtype=np.float64)
    shared["wv"] = np.ascontiguousarray(
        np.einsum("ick,iko->ico", wv64, wap64).astype(BF))
    shared["bap"] = np.ascontiguousarray(b_attn_proj.T, dtype=f)
    shared["wgf"] = np.ascontiguousarray(
        np.tile(wg_final.reshape(3, HD, 3), (1, 1, 43))[:, :, :HD].astype(BF))
    shared["w1"] = np.ascontiguousarray(
        w_mlp1.reshape(3, 3, HD, 1536).transpose(0, 2, 1, 3).astype(BF))
    shared["b1"] = np.ascontiguousarray(
        b_mlp1.reshape(3, 12, HD).transpose(2, 0, 1), dtype=f)
    w2p = np.asarray(w_mlp2, dtype=np.float64) @ np.asarray(w_proj, np.float64)
    shared["w2"] = np.ascontiguousarray(
        w2p.reshape(3, 12, HD, C).transpose(0, 2, 1, 3).astype(BF))
    shared["b2r"] = np.ascontiguousarray(
        (np.asarray(b_mlp2, np.float64) @ np.asarray(w_proj, np.float64))
        .astype(BF))
    shared["bpr"] = np.ascontiguousarray(b_proj.reshape(3, HD).T, dtype=f)

    in_maps = []
    for c in range(N_CORES):
        b, half = c // 2, c % 2
        r0 = half * R
        slab = np.zeros((C, RP, SP), BF)
        glo, ghi = max(0, r0 - 8), min(HH, r0 + R + 8)
        plo = glo - (r0 - 8) + 1
        slab[:, plo:plo + (ghi - glo), 8:SP] = \
            np.asarray(x[b, glo:ghi]).astype(BF).transpose(2, 0, 1)
        m = dict(shared)
        m["xp"] = np.ascontiguousarray(slab)
        in_maps.append(m)
    return in_maps


def kernel(**inputs):
    global _CACHED_NC
    if _CACHED_NC is None:
        _CACHED_NC = build_kernel()
    nc = _CACHED_NC
    in_maps = _prep_inputs(**{k: np.asarray(v) for k, v in inputs.items()})
    res = None
    for attempt in range(3):
        try:
            res = run_bass_kernel_spmd(nc, in_maps,
                                       core_ids=list(range(N_CORES)))
            break
        except Exception:
            if attempt == 2:
                raise
            import time
            time.sleep(2.0)
    out = np.empty((B, HH, WW, C), np.float32)
    for c in range(N_CORES):
        b, half = c // 2, c % 2
        slab = res.results[c]["out_cm"].reshape(C, R, 96)
        out[b, :, half * R:(half + 1) * R, :] = slab.transpose(2, 1, 0)
    return out


# revision 10
# speedup vs baseline: 1.0260x; 1.0150x over previous
"""Trainium2 Bass kernel for nn_MAMoE (conv-MoE -> row attention -> MLP-MoE).

Sharding: 8 cores = (batch b in 0..3) x (H-half in 0..1). All routing is
per-token; the reference's swapaxes(1,2) means attention row r produces
output column w=r, so each core independently computes the full pipeline
for its 48 attention rows and the host reassembles along W.

Layout: padded row stride 104 (8 zero cols serve as both right halo of
row r and left halo of row r+1); conv/gate matmuls use strided rhs APs
([4 rows @ 104, 96]) so no pad columns are ever computed. scores use a
host-fused A = scale * Wq @ Wk^T so only one projection (qh) is needed.
bf16 everywhere with fp32 PSUM accumulation. Phase A is branch-interleaved
and software-pipelined two groups deep (scores of group g-1 and attention
tail of group g-2 are emitted under group g's convs) so the in-order PE
queue never blocks on the ACT/DVE softmax chain. Phase-B weights preload
on the second hardware DMA queue during Phase A.
"""
import numpy as np
import ml_dtypes

import concourse.bass as bass
import concourse.mybir as mybir
import concourse.tile as tile
from concourse import bacc
from concourse.bass_utils import run_bass_kernel_spmd
from concourse.masks import make_identity

F32 = mybir.dt.float32
F32R = mybir.dt.float32r
BF16 = mybir.dt.bfloat16
BF = ml_dtypes.bfloat16

B, HH, WW, C = 4, 96, 96, 384
HD = 128
SCALE = float((HD // 3) ** -0.5)  # 42**-0.5
N_CORES = 8
R = 48            # attention rows per core
RP = 66           # slack row + 8 halo + 48 + 8 halo + 1 slack row
SP = 104          # padded row stride (8 zero pad + 96 valid)
T = R * 96        # tokens per core = 4608
GROUPS = R // 4   # 12 groups of 4 rows
GN = 4 * 96       # tokens per group = 384
# MLP tiles: 8x512 then 2x256 (narrow tail shortens the end-of-kernel drain)
TILES = [(t * 512, 512) for t in range(8)] + [(4096, 256), (4352, 256)]

TAPS_A = [
    [(dr, ds) for dr in (-1, 0, 1) for ds in (-1, 0, 1)],
    [(dr, 0) for dr in range(-4, 5)],
    [(0, ds) for ds in range(-4, 5)],
]
TAPS_B = [
    [(dr, ds) for dr in (-2, 0, 2) for ds in (-2, 0, 2)],
    [(dr, 0) for dr in range(-8, 9, 2)],
    [(0, ds) for ds in range(-8, 9, 2)],
]

_CACHED_NC = None


def build_kernel():
    nc = bacc.Bacc("TRN2", target_bir_lowering=False, debug=False)

    xp = nc.dram_tensor("xp", [C, RP, SP], BF16, kind="ExternalInput").ap()
    wca = nc.dram_tensor("wca", [3, HD, 9, HD], BF16, kind="ExternalInput").ap()
    wcb = nc.dram_tensor("wcb", [3, HD, 9, HD], BF16, kind="ExternalInput").ap()
    bca = nc.dram_tensor("bca", [HD, 3], F32, kind="ExternalInput").ap()
    bcb = nc.dram_tensor("bcb", [HD, 3], F32, kind="ExternalInput").ap()
    wgd = nc.dram_tensor("wgd", [3, HD, HD], BF16, kind="ExternalInput").ap()
    eb3 = nc.dram_tensor("eb3", [3, 384], BF16, kind="ExternalInput").ap()
    wqh = nc.dram_tensor("wqh", [3, HD, HD], BF16, kind="ExternalInput").ap()
    wv = nc.dram_tensor("wv", [3, HD, HD], BF16, kind="ExternalInput").ap()
    bap = nc.dram_tensor("bap", [HD, 3], F32, kind="ExternalInput").ap()
    wgf = nc.dram_tensor("wgf", [3, HD, HD], BF16, kind="ExternalInput").ap()
    w1 = nc.dram_tensor("w1", [3, HD, 3, 1536], BF16, kind="ExternalInput").ap()
    b1 = nc.dram_tensor("b1", [HD, 3, 12], F32, kind="ExternalInput").ap()
    w2 = nc.dram_tensor("w2", [3, HD, 12, C], BF16, kind="ExternalInput").ap()
    b2r = nc.dram_tensor("b2r", [3, C], BF16, kind="ExternalInput").ap()
    bpr = nc.dram_tensor("bpr", [HD, 3], F32, kind="ExternalInput").ap()
    out_cm = nc.dram_tensor("out_cm", [C, T], F32, kind="ExternalOutput").ap()

    with tile.TileContext(nc) as tc:
        with tc.tile_pool(name="consts", bufs=1) as consts, \
             tc.tile_pool(name="persist", bufs=1) as persist:
            ident = consts.tile([HD, HD], F32)
            make_identity(nc, ident)
            identb = consts.tile([HD, HD], BF16)
            nc.vector.tensor_copy(identb, ident)

            bca_sb = persist.tile([HD, 3], F32)
            bcb_sb = persist.tile([HD, 3], F32)
            bap_sb = persist.tile([HD, 3], F32)

            xc_t = [persist.tile([HD, T], BF16, tag=f"xc{i}", name=f"xc{i}") for i in range(3)]

            # Phase-B weights (DMAs issued later, on the scalar HWDGE queue)
            b1_sb = persist.tile([HD, 3, 12], F32)
            b2r_sb = persist.tile([3, C], BF16)
            wgf_sb = persist.tile([HD, 3, HD], BF16)
            bpr_sb = persist.tile([HD, 3], F32)
            eb3_sb = persist.tile([3, 384], BF16)
            w1_sb = [persist.tile([HD, 3, 1536], BF16, tag=f"w1_{e}", name=f"w1_{e}")
                     for e in range(3)]
            w2_sb = [persist.tile([HD, 12, C], BF16, tag=f"w2_{e}", name=f"w2_{e}")
                     for e in range(3)]

            phase_b_loads = []
            for e in range(3):
                phase_b_loads.append((w1_sb[e], w1[e]))
                phase_b_loads.append((w2_sb[e], w2[e]))
            phase_b_loads += [
                (b1_sb, b1), (b2r_sb, b2r), (bpr_sb, bpr), (eb3_sb, eb3),
            ]

            # ---------------- Phase A: conv MoE + attention, interleaved --
            with tc.tile_pool(name="xw", bufs=1) as xw, \
                 tc.tile_pool(name="gp3", bufs=3) as gp3, \
                 tc.tile_pool(name="gp6", bufs=6) as gp6, \
                 tc.tile_pool(name="ap3", bufs=3) as ap3, \
                 tc.tile_pool(name="ap9", bufs=9) as ap9, \
                 tc.tile_pool(name="psC", bufs=3, space="PSUM") as psC, \
                 tc.tile_pool(name="psT", bufs=5, space="PSUM") as psT:
                xp_sb = [xw.tile([HD, RP, SP], BF16, tag=f"xp{i}", name=f"xp{i}")
                         for i in range(3)]
                wgd_sb, wca_sb, wcb_sb, wqh_sb, wv_sb = [], [], [], [], []
                for i in range(3):
                    wgd_sb.append(xw.tile([HD, HD], BF16, tag=f"wgd{i}", name=f"wgd{i}"))
                    wca_sb.append(xw.tile([HD, 9, HD], BF16, tag=f"wca{i}", name=f"wca{i}"))
                    wcb_sb.append(xw.tile([HD, 9, HD], BF16, tag=f"wcb{i}", name=f"wcb{i}"))
                    wqh_sb.append(xw.tile([HD, HD], BF16, tag=f"wqh{i}", name=f"wqh{i}"))
                    wv_sb.append(xw.tile([HD, HD], BF16, tag=f"wv{i}", name=f"wv{i}"))

                # Criticality-ordered DMA issue across both HWDGE queues.
                nc.sync.dma_start(out=xp_sb[0][:, :25, :], in_=xp[0:HD, :25, :])
                for i in range(3):
                    nc.sync.dma_start(out=wgd_sb[i], in_=wgd[i])
                    nc.sync.dma_start(out=wca_sb[i], in_=wca[i])
                    nc.sync.dma_start(out=wcb_sb[i], in_=wcb[i])
                nc.scalar.dma_start(out=xp_sb[1][:, :25, :],
                                    in_=xp[HD:2 * HD, :25, :])
                nc.scalar.dma_start(out=xp_sb[2][:, :25, :],
                                    in_=xp[2 * HD:3 * HD, :25, :])
                nc.scalar.dma_start(out=bca_sb, in_=bca)
                nc.scalar.dma_start(out=bcb_sb, in_=bcb)
                nc.scalar.dma_start(out=bap_sb, in_=bap)
                for i in range(3):
                    nc.sync.dma_start(out=wqh_sb[i], in_=wqh[i])
                    nc.sync.dma_start(out=wv_sb[i], in_=wv[i])
                for i in range(3):
                    nc.sync.dma_start(out=xp_sb[i][:, 25:45, :],
                                      in_=xp[i * HD:(i + 1) * HD, 25:45, :])
                for i in range(3):
                    nc.sync.dma_start(out=xp_sb[i][:, 45:, :],
                                      in_=xp[i * HD:(i + 1) * HD, 45:, :])
                nc.sync.dma_start(out=wgf_sb, in_=wgf.rearrange("a p b -> p a b"))

                xpf = [xp_sb[i].rearrange("p r s -> p (r s)") for i in range(3)]

                def win(i, g, dr, ds):
                    """[128, 4, 96] strided window: rows (9+4g+dr).., col 8+ds."""
                    base = (9 + 4 * g + dr) * SP + 8 + ds
                    return xpf[i][:, base:base + 4 * SP] \
                        .rearrange("p (r s) -> p r s", s=SP)[:, :, :96]

                def stage1(i, g):
                    """gate + both expert convs + moe blend for (branch, group)."""
                    plg = psC.tile([HD, GN], F32, tag="ps")
                    nc.tensor.matmul(plg, wgd_sb[i], win(i, g, 0, 0),
                                     start=True, stop=True)
                    ex = gp3.tile([HD, GN], BF16, tag="ex")
                    nc.scalar.activation(ex, plg,
                                         mybir.ActivationFunctionType.Tanh,
                                         scale=-0.5)
                    pa = psC.tile([HD, GN], F32, tag="ps")
                    for ti, (dr, ds) in enumerate(TAPS_A[i]):
                        nc.tensor.matmul(pa, wca_sb[i][:, ti, :], win(i, g, dr, ds),
                                         start=(ti == 0), stop=(ti == 8))
                    pb = psC.tile([HD, GN], F32, tag="ps")
                    for ti, (dr, ds) in enumerate(TAPS_B[i]):
                        nc.tensor.matmul(pb, wcb_sb[i][:, ti, :], win(i, g, dr, ds),
                                         start=(ti == 0), stop=(ti == 8))
                    # moe = g0*(ca - cb) + cb  (bias-add fused on ACT)
                    ca = gp3.tile([HD, GN], BF16, tag="ca")
                    nc.scalar.activation(ca, pa,
                                         mybir.ActivationFunctionType.Identity,
                                         bias=bca_sb[:, i:i + 1], scale=0.5)
                    cb = gp3.tile([HD, GN], BF16, tag="cb")
                    nc.scalar.activation(cb, pb,
                                         mybir.ActivationFunctionType.Identity,
                                         bias=bcb_sb[:, i:i + 1], scale=0.5)
                    dd = gp3.tile([HD, GN], BF16, tag="dd")
                    nc.vector.tensor_sub(dd, ca, cb)
                    d2 = gp3.tile([HD, GN], BF16, tag="d2")
                    nc.vector.tensor_mul(d2, dd, ex)
                    ss = gp3.tile([HD, GN], BF16, tag="ss")
                    nc.vector.tensor_add(ss, ca, cb)
                    moe = gp6.tile([HD, GN], BF16, tag="moe")
                    nc.vector.tensor_add(moe, ss, d2)
                    return moe

                def stage2(i, moe):
                    """fused qh = (scale*Wq@Wk^T)^T moe and v (w/ proj fused)."""
                    pqh = psT.tile([HD, GN], F32, tag="ps")
                    nc.tensor.matmul(pqh, wqh_sb[i], moe, start=True, stop=True)
                    qh = gp6.tile([HD, GN], BF16, tag="qh")
                    nc.scalar.copy(qh, pqh)
                    pvt = psT.tile([96, 4 * HD], F32, tag="ps")
                    for j in range(4):
                        nc.tensor.matmul(pvt[:, j * HD:(j + 1) * HD],
                                         moe[:, j * 96:(j + 1) * 96],
                                         wv_sb[i], start=True, stop=True)
                    vt_sb = ap9.tile([96, 4 * HD], BF16, tag="vt")
                    nc.vector.tensor_copy(vt_sb, pvt)
                    return qh, vt_sb

                def stage3(qh, moe):
                    """scores + softmax numerator/denominator."""
                    psc = psT.tile([96, GN], F32, tag="ps")
                    for j in range(4):
                        nc.tensor.matmul(psc[:, j * 96:(j + 1) * 96],
                                         qh[:, j * 96:(j + 1) * 96],
                                         moe[:, j * 96:(j + 1) * 96],
                                         start=True, stop=True)
                    probs = ap3.tile([96, GN], BF16, tag="probs")
                    nc.scalar.activation(probs, psc,
                                         mybir.ActivationFunctionType.Exp)
                    zsum = ap3.tile([96, 4], F32, tag="zsum")
                    nc.vector.tensor_reduce(
                        zsum, probs.rearrange("p (j q) -> p j q", q=96),
                        axis=mybir.AxisListType.X, op=mybir.AluOpType.add)
                    rec = ap3.tile([96, 4], F32, tag="rec")
                    nc.vector.reciprocal(rec, zsum)
                    pn = gp6.tile([96, GN], BF16, tag="pn")
                    for j in range(4):
                        nc.vector.tensor_scalar_mul(
                            pn[:, j * 96:(j + 1) * 96],
                            probs[:, j * 96:(j + 1) * 96], rec[:, j:j + 1])
                    return pn

                def stage4(i, g, pn, vt_sb):
                    """probs transpose + attention output + xc write."""
                    ppt = psT.tile([96, GN], BF16, tag="ps")
                    for j in range(4):
                        nc.tensor.transpose(ppt[:, j * 96:(j + 1) * 96],
                                            pn[:, j * 96:(j + 1) * 96],
                                            identb[:96, :96])
                    pt_sb = ap3.tile([96, GN], BF16, tag="pt")
                    nc.vector.tensor_copy(pt_sb, ppt)
                    po = psT.tile([HD, GN], F32, tag="ps")
                    for j in range(4):
                        nc.tensor.matmul(po[:, j * 96:(j + 1) * 96],
                                         vt_sb[:, j * HD:(j + 1) * HD],
                                         pt_sb[:, j * 96:(j + 1) * 96],
                                         start=True, stop=True)
                    nc.vector.tensor_scalar(
                        xc_t[i][:, g * GN:(g + 1) * GN], po,
                        bap_sb[:, i:i + 1], None, op0=mybir.AluOpType.add)

                s2 = {}  # g -> [(qh, vt), ...]; smoe: g -> [moe, ...]
                smoe = {}
                s3 = {}  # g -> [pn, ...]
                for g in range(GROUPS):
                    moes = []
                    for i in range(3):
                        moes.append(stage1(i, g))
                        if i == 0 and g - 1 in s2:
                            s3[g - 1] = [stage3(s2[g - 1][ii][0],
                                                smoe[g - 1][ii])
                                         for ii in range(3)]
                        if i == 1 and g - 2 in s3:
                            for ii in range(3):
                                stage4(ii, g - 2, s3[g - 2][ii],
                                       s2[g - 2][ii][1])
                            del s3[g - 2], s2[g - 2], smoe[g - 2]
                    # stream Phase-B weights on the scalar queue mid-phase
                    if 2 <= g <= 6:
                        for dst, src in phase_b_loads[2 * (g - 2):2 * (g - 1)]:
                            nc.scalar.dma_start(out=dst, in_=src)
                    s2[g] = [stage2(i, moes[i]) for i in range(3)]
                    smoe[g] = moes
                g = GROUPS
                s3[g - 1] = [stage3(s2[g - 1][ii][0], smoe[g - 1][ii])
                             for ii in range(3)]
                for gg in (g - 2, g - 1):
                    for ii in range(3):
                        stage4(ii, gg, s3[gg][ii], s2[gg][ii][1])

            # ---------------- Phase B: final MLP MoE + proj ---------------
            with tc.tile_pool(name="bpool", bufs=3) as bpool, \
                 tc.tile_pool(name="gpoolB", bufs=2) as gpoolB, \
                 tc.tile_pool(name="psL", bufs=3, space="PSUM") as psL, \
                 tc.tile_pool(name="psGB", bufs=1, space="PSUM") as psGB, \
                 tc.tile_pool(name="psPG", bufs=1, space="PSUM") as psPG, \
                 tc.tile_pool(name="psB", bufs=3, space="PSUM") as psB:

                def gating_part1a(t):
                    """logits matmul for tile t."""
                    t0, nt = TILES[t]
                    plg = psGB.tile([3, 512], F32, tag="ps", name="plg")
                    for kc in range(3):
                        nc.tensor.matmul(plg[:, :nt], wgf_sb[:, kc, :3],
                                         xc_t[kc][:, t0:t0 + nt],
                                         start=(kc == 0), stop=(kc == 2))
                    lsb = gpoolB.tile([3, 512], F32, tag="lsb", name="lsb")
                    nc.vector.tensor_copy(lsb[:, :nt], plg[:, :nt])
                    return lsb

                def gating_part1b(lsb, t):
                    """token-major top-2 softmax math."""
                    nt = TILES[t][1]
                    n4 = nt // HD  # 4 or 2 column-blocks of 128 tokens
                    plt = psGB.tile([HD, 12], F32, tag="ps", name="plt")
                    for t4 in range(n4):
                        nc.tensor.transpose(plt[:, t4 * 3:(t4 + 1) * 3],
                                            lsb[:, t4 * HD:(t4 + 1) * HD],
                                            ident[:3, :3])
                    lt = gpoolB.tile([HD, 12], F32, tag="lt", name="lt")
                    nc.vector.tensor_copy(lt[:, :3 * n4], plt[:, :3 * n4])
                    l3 = lt.rearrange("p (j e) -> p j e", e=3)
                    mx = gpoolB.tile([HD, 4], F32, tag="mx", name="mx")
                    nc.vector.tensor_reduce(mx[:, :n4], l3[:, :n4],
                                            axis=mybir.AxisListType.X,
                                            op=mybir.AluOpType.max)
                    mn = gpoolB.tile([HD, 4], F32, tag="mn", name="mn")
                    nc.vector.tensor_reduce(mn[:, :n4], l3[:, :n4],
                                            axis=mybir.AxisListType.X,
                                            op=mybir.AluOpType.min)
                    sm = gpoolB.tile([HD, 4], F32, tag="sm", name="sm")
                    nc.vector.tensor_reduce(sm[:, :n4], l3[:, :n4],
                                            axis=mybir.AxisListType.X,
                                            op=mybir.AluOpType.add)
                    t1 = gpoolB.tile([HD, 4], F32, tag="t1", name="t1")
                    nc.vector.tensor_sub(t1[:, :n4], sm[:, :n4], mx[:, :n4])
                    mid = gpoolB.tile([HD, 4], F32, tag="mid", name="mid")
                    nc.vector.tensor_sub(mid[:, :n4], t1[:, :n4], mn[:, :n4])
                    dm = gpoolB.tile([HD, 4], F32, tag="dm", name="dm")
                    nc.vector.tensor_sub(dm[:, :n4], mx[:, :n4], mid[:, :n4])
                    th = gpoolB.tile([HD, 4], F32, tag="th", name="th")
                    nc.scalar.activation(th[:, :n4], dm[:, :n4],
                                         mybir.ActivationFunctionType.Tanh,
                                         scale=0.5)
                    gmx = gpoolB.tile([HD, 4], F32, tag="gmx", name="gmx")
                    nc.vector.tensor_scalar(gmx[:, :n4], th[:, :n4], 0.5, 0.5,
                                            op0=mybir.AluOpType.mult,
                                            op1=mybir.AluOpType.add)
                    eqx = gpoolB.tile([HD, 12], F32, tag="eqx", name="eqx")
                    eqn = gpoolB.tile([HD, 12], F32, tag="eqn", name="eqn")
                    for t4 in range(n4):
                        sl = slice(t4 * 3, (t4 + 1) * 3)
                        nc.vector.tensor_scalar(eqx[:, sl], lt[:, sl],
                                                mx[:, t4:t4 + 1], None,
                                                op0=mybir.AluOpType.is_equal)
                        nc.vector.tensor_scalar(eqn[:, sl], lt[:, sl],
                                                mn[:, t4:t4 + 1], None,
                                                op0=mybir.AluOpType.is_equal)
                    # u = 1 - eqx - eqn (mid indicator); g = gmx*(eqx-u) + u
                    s1 = gpoolB.tile([HD, 12], F32, tag="s1", name="s1")
                    nc.vector.tensor_add(s1[:, :3 * n4], eqx[:, :3 * n4],
                                         eqn[:, :3 * n4])
                    u = gpoolB.tile([HD, 12], F32, tag="u", name="u")
                    nc.vector.tensor_scalar(u[:, :3 * n4], s1[:, :3 * n4],
                                            -1.0, 1.0,
                                            op0=mybir.AluOpType.mult,
                                            op1=mybir.AluOpType.add)
                    d0 = gpoolB.tile([HD, 12], F32, tag="d0", name="d0")
                    nc.vector.tensor_sub(d0[:, :3 * n4], eqx[:, :3 * n4],
                                         u[:, :3 * n4])
                    p0 = gpoolB.tile([HD, 12], F32, tag="p0", name="p0")
                    for t4 in range(n4):
                        sl = slice(t4 * 3, (t4 + 1) * 3)
                        nc.vector.tensor_scalar_mul(p0[:, sl], d0[:, sl],
                                                    gmx[:, t4:t4 + 1])
                    gm = gpoolB.tile([HD, 12], BF16, tag="gm", name="gm")
                    nc.vector.tensor_add(gm[:, :3 * n4], p0[:, :3 * n4],
                                         u[:, :3 * n4])
                    return gm

                def gating_part2(gm, t):
                    """expert-major gates [3, nt] from token-major gm."""
                    nt = TILES[t][1]
                    n4 = nt // HD
                    pgt = psGB.tile([3, 512], BF16, tag="ps", name="pgt")
                    for t4 in range(n4):
                        nc.tensor.transpose(pgt[:, t4 * HD:(t4 + 1) * HD],
                                            gm[:, t4 * 3:(t4 + 1) * 3],
                                            identb)
                    gates_r = gpoolB.tile([3, 512], BF16, tag="gates",
                                          name="gates_r")
                    nc.scalar.copy(gates_r[:, :nt], pgt[:, :nt])
                    return gates_r

                ntl = len(TILES)
                gates_cur = gating_part2(
                    gating_part1b(gating_part1a(0), 0), 0)
                lsb_next = gating_part1a(1)
                gm_next = None
                gates_next = None
                for t in range(ntl):
                    t0, nt = TILES[t]
                    gts = gates_cur[:, :nt]
                    pd = [psL.tile([HD, 512], F32, tag="down", name=f"pd{_i}") for _i in range(3)]
                    for mp in range(3):
                        nc.tensor.matmul(pd[mp][:, :nt],
                                         b2r_sb[:, mp * HD:(mp + 1) * HD],
                                         gts, start=True, stop=False)
                    for e in range(3):
                        if e == 1 and t + 1 < ntl:
                            gm_next = gating_part1b(lsb_next, t + 1)
                        if e == 2 and t + 1 < ntl:
                            gates_next = gating_part2(gm_next, t + 1)
                            lsb_next = gating_part1a(t + 2) if t + 2 < ntl \
                                else None
                        pgb = psPG.tile([HD, 512], F32, tag="pgb", name="pgb")

                        def down(m, hs):
                            for mp in range(3):
                                nc.tensor.matmul(
                                    pd[mp][:, :nt],
                                    w2_sb[e][:, m, mp * HD:(mp + 1) * HD],
                                    hs[:, :nt], start=False,
                                    stop=(e == 2 and m == 11))

                        hsq = []  # (m, hs) pending down-projections
                        for m in range(12):
                            pu = psB.tile([HD, 512], F32, tag="ps", name="pu")
                            for kc in range(3):
                                nc.tensor.matmul(
                                    pu[:, :nt],
                                    w1_sb[e][:, kc, m * HD:(m + 1) * HD],
                                    xc_t[kc][:, t0:t0 + nt],
                                    start=(kc == 0), stop=(kc == 2))
                            if m == 0:
                                nc.tensor.matmul(
                                    pgb[:, :nt],
                                    eb3_sb[:, e * HD:(e + 1) * HD],
                                    gts, start=True, stop=True)
                            h = bpool.tile([HD, 512], F32, tag="h")
                            nc.scalar.activation(
                                h[:, :nt], pu[:, :nt],
                                mybir.ActivationFunctionType.Gelu,
                                bias=b1_sb[:, e, m:m + 1])
                            hs = bpool.tile([HD, 512], BF16, tag="hs")
                            nc.vector.tensor_mul(hs[:, :nt], h[:, :nt],
                                                 pgb[:, :nt])
                            hsq.append((m, hs))
                            if m >= 2:
                                down(*hsq.pop(0))
                        for pair in hsq:
                            down(*pair)
                    gates_cur = gates_next
                    for mp in range(3):
                        osb = bpool.tile([HD, 512], F32, tag="osb")
                        nc.scalar.activation(osb[:, :nt], pd[mp][:, :nt],
                                             mybir.ActivationFunctionType.Identity,
                                             bias=bpr_sb[:, mp:mp + 1])
                        nc.sync.dma_start(
                            out=out_cm[mp * HD:(mp + 1) * HD, t0:t0 + nt],
                            in_=osb[:, :nt])
    nc.compile()
    return nc


def _prep_inputs(x, w_e1, b_e1, w_e2, b_e2, w_e3, b_e3, w_e4, b_e4, w_e5, b_e5,
                 w_e6, b_e6, wg1, wg2, wg3, w_qkv, w_attn_proj, b_attn_proj,
                 wg_final, w_mlp1, b_mlp1, w_mlp2, b_mlp2, w_proj, b_proj):
    f = np.float32
    shared = {}
    # conv weights pre-transposed to [cin(p), tap, cout] for contiguous DMA
    shared["wca"] = np.ascontiguousarray(np.stack([
        w_e1.reshape(9, HD, HD).transpose(1, 0, 2),
        w_e3.reshape(9, HD, HD).transpose(1, 0, 2),
        w_e5.reshape(9, HD, HD).transpose(1, 0, 2)]).astype(BF))
    shared["wcb"] = np.ascontiguousarray(np.stack([
        w_e2.reshape(9, HD, HD).transpose(1, 0, 2),
        w_e4.reshape(9, HD, HD).transpose(1, 0, 2),
        w_e6.reshape(9, HD, HD).transpose(1, 0, 2)]).astype(BF))
    shared["bca"] = np.ascontiguousarray(
        np.stack([b_e1, b_e3, b_e5], axis=1) * 0.5, dtype=f)
    shared["bcb"] = np.ascontiguousarray(
        np.stack([b_e2, b_e4, b_e6], axis=1) * 0.5, dtype=f)
    wgs = np.stack([wg1, wg2, wg3])
    shared["wgd"] = np.ascontiguousarray(
        np.repeat((wgs[:, :, 1] - wgs[:, :, 0])[:, :, None], HD, axis=2)
        .astype(BF))
    eb3 = np.zeros((3, 384), f)
    for e in range(3):
        eb3[e, e * 128:(e + 1) * 128] = 1.0
    shared["eb3"] = eb3.astype(BF)
    # fused score matrix A = SCALE * Wq @ Wk^T  (scores = moe A moe^T)
    wq64 = np.asarray(w_qkv[:, :, :HD], dtype=np.float64)
    wk64 = np.asarray(w_qkv[:, :, HD:2 * HD], dtype=np.float64)
    shared["wqh"] = np.ascontiguousarray(
        (SCALE * np.einsum("iac,ibc->iab", wq64, wk64)).astype(BF))
    wv64 = np.asarray(w_qkv[:, :, 2 * HD:], dtype=np.float64)
    wap64 = np.asarray(w_attn_proj, dtype=np.float64)
    shared["wv"] = np.ascontiguousarray(
        np.einsum("ick,iko->ico", wv64, wap64).astype(BF))
    shared["bap"] = np.ascontiguousarray(b_attn_proj.T, dtype=f)
    shared["wgf"] = np.ascontiguousarray(
        np.tile(wg_final.reshape(3, HD, 3), (1, 1, 43))[:, :, :HD].astype(BF))
    shared["w1"] = np.ascontiguousarray(
        w_mlp1.reshape(3, 3, HD, 1536).transpose(0, 2, 1, 3).astype(BF))
    shared["b1"] = np.ascontiguousarray(
        b_mlp1.reshape(3, 12, HD).transpose(2, 0, 1), dtype=f)
    w2p = np.asarray(w_mlp2, dtype=np.float64) @ np.asarray(w_proj, np.float64)
    shared["w2"] = np.ascontiguousarray(
        w2p.reshape(3, 12, HD, C).transpose(0, 2, 1, 3).astype(BF))
    shared["b2r"] = np.ascontiguousarray(
        (np.asarray(b_mlp2, np.float64) @ np.asarray(w_proj, np.float64))
        .astype(BF))
    shared["bpr"] = np.ascontiguousarray(b_proj.reshape(3, HD).T, dtype=f)

    in_maps = []
    for c in range(N_CORES):
        b, half = c // 2, c % 2
        r0 = half * R
        slab = np.zeros((C, RP, SP), BF)
        glo, ghi = max(0, r0 - 8), min(HH, r0 + R + 8)
        plo = glo - (r0 - 8) + 1
        slab[:, plo:plo + (ghi - glo), 8:SP] = \
            np.asarray(x[b, glo:ghi]).astype(BF).transpose(2, 0, 1)
        m = dict(shared)
        m["xp"] = np.ascontiguousarray(slab)
        in_maps.append(m)
    return in_maps


def kernel(**inputs):
    global _CACHED_NC
    if _CACHED_NC is None:
        _CACHED_NC = build_kernel()
    nc = _CACHED_NC
    in_maps = _prep_inputs(**{k: np.asarray(v) for k, v in inputs.items()})
    res = None
    for attempt in range(3):
        try:
            res = run_bass_kernel_spmd(nc, in_maps,
                                       core_ids=list(range(N_CORES)))
            break
        except Exception:
            if attempt == 2:
                raise
            import time
            time.sleep(2.0)
    out = np.empty((B, HH, WW, C), np.float32)
    for c in range(N_CORES):
        b, half = c // 2, c % 2
        slab = res.results[c]["out_cm"].reshape(C, R, 96)
        out[b, :, half * R:(half + 1) * R, :] = slab.transpose(2, 1, 0)
    return out
